# revision 28
# baseline (speedup 1.0000x reference)
"""Trainium2 Bass kernel for the ACOPF bus-embedder GNN (TransformerConv x L
layers + per-type dense head), distributed over 8 NeuronCores.

Sharding: core c handles node type t = c // 2 and column-half h = c % 2 of the
per-type fc weight.  The message passing is graph-partitioned by target node
on the host (edges sorted by target, slotted to fixed degree PAD); each core
pair duplicates it, so no collectives are needed.

Key algebraic reduction (C == 2): with q = x@Wq+bq, k = xs@Wk+bk+ea@We,
v = xs@Wv+bv+ea@We, the attention logit  alpha_e = q[tgt]*k_e / sqrt(H)
collapses to   alpha_e = p0*xs0 + p1*xs1 + p2*ea0 + p3*ea1 + p4
with per-target scalars p_j affine in x[tgt] (weight products folded on the
host, the affine map evaluated on-device).  The message is likewise affine in
the softmax-weighted sums r_j = sum_e a_e*plane_j, so the per-edge pipeline is
a few fused multiply-adds plus one exp.

Device mapping:
  - edge planes (xs0/xs1/ea0/ea1/mask) are host-slotted and DMA'd in bf16.
  - alpha / exp / weighted reductions run on DVE + Act, batched per group of
    4 node tiles to amortize instruction overhead.
  - the fc GEMV y = Wfc @ val streams the bf16 weight matrix over all three
    DMA-capable queues (sync/HWDGE, scalar/HWDGE, gpsimd/SWDGE) concurrently
    and feeds the PE with the weights as the *stationary* operand (ldweights),
    val as the moving operand, accumulating [128,1] psum columns - the PE cost
    of a GEMV is set by weight ingest, and ldweights overlaps the stream.
"""

import numpy as np
import ml_dtypes

from concourse import bass, bacc, mybir
import concourse.tile as tile
from concourse.bass_utils import run_bass_kernel_spmd

F32 = mybir.dt.float32
BF16 = mybir.dt.bfloat16
NPBF = ml_dtypes.bfloat16
ALU = mybir.AluOpType
ACTF = mybir.ActivationFunctionType

LAST = {}  # filled with exec info for test harnesses

NEG_BIG = -1.0e30


def _host_prep(x, edge_attr, src_idx, tgt_idx, Wq, bq, Wk, bk, Wv, bv, We,
               Wskip, bskip, Wfc, bfc):
    """Build per-core input maps + dims. All index juggling happens here."""
    T, N, C = x.shape
    E = src_idx.shape[1]
    L, H = Wq.shape[1], Wq.shape[3]
    K = Wfc.shape[2]
    assert T == 4, "sharding assumes 4 node types x 2 col-halves = 8 cores"
    assert C == 2 and K == 2 * L * N, (C, K, L, N)
    assert N % 128 == 0
    NT = N // 128
    TN = T * N
    KT = K // 128

    x_all = np.asarray(x, np.float32).reshape(TN, C)

    # --- per-type graph partitioning (sort by target, slot to fixed degree)
    per_type = []
    degmax = 0
    for t in range(T):
        tgt = np.asarray(tgt_idx[t])
        order = np.argsort(tgt, kind="stable")
        src_s = np.asarray(src_idx[t])[order]
        tgt_s = tgt[order]
        ea_s = np.asarray(edge_attr[t], np.float32)[order]
        deg = np.bincount(tgt_s, minlength=N)
        degmax = max(degmax, int(deg.max()))
        per_type.append((src_s, tgt_s, ea_s, deg))

    PAD = int(max(16, -(-degmax // 8) * 8))  # round up to mult of 8

    half = H // 2
    core_inputs = []
    for t in range(T):
        src_s, tgt_s, ea_s, deg = per_type[t]
        starts = np.zeros(N + 1, np.int64)
        np.cumsum(deg, out=starts[1:])
        pos = np.arange(E) - starts[tgt_s]

        # planes [5][NT][128][PAD] -> [128, 5*NT*PAD] bf16
        # j: 0=xs0 1=xs1 2=ea0 3=ea1 4=mask(-BIG at empty slots)
        pls = np.zeros((N, PAD, 4), np.float32)
        xs_e = x_all[src_s]
        pls[tgt_s, pos, 0] = xs_e[:, 0]
        pls[tgt_s, pos, 1] = xs_e[:, 1]
        pls[tgt_s, pos, 2] = ea_s[:, 0]
        pls[tgt_s, pos, 3] = ea_s[:, 1]
        valid = np.zeros((N, PAD), bool)
        valid[tgt_s, pos] = True
        mneg = np.where(valid, 0.0, NEG_BIG).astype(np.float32)
        allp = np.concatenate([pls, mneg[:, :, None]], axis=2)   # [N, PAD, 5]
        planes = (allp.reshape(NT, 128, PAD, 5)
                  .transpose(1, 3, 0, 2)           # [128, 5, NT, PAD]
                  .reshape(128, 5 * NT * PAD)).astype(NPBF)

        xt = np.asarray(x[t], np.float32)
        xpl = (xt.reshape(NT, 128, 2).transpose(1, 2, 0)
               .reshape(128, 2 * NT))              # [128, (c, NT)] c-major

        # phi constants: alpha = (x@A + c) . [xs0,xs1,ea0,ea1,1] / sqrt(H)
        phic = np.zeros((L, 15), np.float32)
        pqc = np.zeros((L, 16), np.float32)
        s = 1.0 / np.sqrt(H)
        for l in range(L):
            wq, bq_ = np.asarray(Wq[t, l], np.float64), np.asarray(bq[t, l], np.float64)
            wk, bk_ = np.asarray(Wk[t, l], np.float64), np.asarray(bk[t, l], np.float64)
            wv, bv_ = np.asarray(Wv[t, l], np.float64), np.asarray(bv[t, l], np.float64)
            we = np.asarray(We[t, l], np.float64)
            wsk, bsk_ = np.asarray(Wskip[t, l], np.float64), np.asarray(bskip[t, l], np.float64)
            A = np.concatenate([wq @ wk.T, wq @ we.T, (wq @ bk_)[:, None]], 1) * s  # [2,5]
            cvec = np.concatenate([bq_ @ wk.T, bq_ @ we.T, [bq_ @ bk_]]) * s        # [5]
            phic[l, 0:5] = A[0]
            phic[l, 5:10] = A[1]
            phic[l, 10:15] = cvec
            for qi, sl in enumerate((slice(0, half), slice(half, H))):
                pqc[l, 8 * qi:8 * qi + 8] = np.concatenate([
                    wv[:, sl].sum(1), we[:, sl].sum(1), [bv_[sl].sum()],
                    wsk[:, sl].sum(1), [bsk_[sl].sum()]])

        # Wfc rows permuted to k' = ((jt*L + l)*2 + q)*128 + p so val columns
        # are produced tile-major (original k = l*2N + 256*jt + 2*p + q).
        wt_full = np.asarray(Wfc[t], np.float32).T            # [K, 2N], k order
        wt_full = (wt_full.reshape(L, NT, 128, 2, 2 * N)
                   .transpose(1, 0, 3, 2, 4)
                   .reshape(K, 2 * N))                        # k' order

        for h in range(2):
            # consts: [xpl 2NT][phic 15L][pqc 16L][bfc2 16]
            bfc2 = (np.asarray(bfc[t, h * N:(h + 1) * N], np.float32)
                    .reshape(NT, 128).T)                      # [128, nb]
            consts = np.concatenate([
                xpl,
                np.tile(phic.reshape(1, 15 * L), (128, 1)),
                np.tile(pqc.reshape(1, 16 * L), (128, 1)),
                bfc2,
            ], axis=1)
            core_inputs.append({
                "planes": np.ascontiguousarray(planes),
                "consts": np.ascontiguousarray(consts.astype(np.float32)),
                "wt": np.ascontiguousarray(
                    wt_full[:, h * N:(h + 1) * N]).astype(NPBF),
            })

    dims = dict(T=T, N=N, C=C, E=E, L=L, H=H, K=K, NT=NT, PAD=PAD, TN=TN,
                KT=KT)
    return core_inputs, dims


def _build_graph(dims, debug=False):
    N, L, K = dims["N"], dims["L"], dims["K"]
    NT, PAD, KT = dims["NT"], dims["PAD"], dims["KT"]
    NB = N // 128              # psum column blocks
    CN = 2 * NT + 31 * L + NB  # consts width
    GS = 4                     # node tiles per message-passing group
    NG = NT // GS              # number of groups
    KTPER = 4                  # k tiles per weight DMA chunk (2 MiB)
    NCH = KT // KTPER          # weight chunks
    KPG = KT // NG             # ktiles per group

    nc = bacc.Bacc()
    planes_d = nc.declare_dram_parameter("planes", [128, 5 * NT * PAD], BF16,
                                         isOutput=False)
    consts_d = nc.declare_dram_parameter("consts", [128, CN], F32,
                                         isOutput=False)
    wt_d = nc.declare_dram_parameter("wt", [K, N], BF16, isOutput=False)
    out_d = nc.declare_dram_parameter("out", [128, NB], F32, isOutput=True)
    if debug:
        dbg_phi = nc.declare_dram_parameter("dbg_phi", [128, L * 5 * NT], F32,
                                            isOutput=True)
        dbg_s = nc.declare_dram_parameter("dbg_s", [128, L * NT], F32,
                                          isOutput=True)
        dbg_rr = nc.declare_dram_parameter("dbg_rr", [128, 4 * L * NT], F32,
                                           isOutput=True)
        dbg_val = nc.declare_dram_parameter("dbg_val", [128, KT], F32,
                                            isOutput=True)

    with tile.TileContext(nc) as tc:
        import contextlib
        with contextlib.ExitStack() as ctx:
            persist = ctx.enter_context(tc.tile_pool(name="persist", bufs=1))
            wS = ctx.enter_context(tc.tile_pool(name="wS", bufs=3))
            wA = ctx.enter_context(tc.tile_pool(name="wA", bufs=3))
            wP = ctx.enter_context(tc.tile_pool(name="wP", bufs=3))
            work = ctx.enter_context(tc.tile_pool(name="work", bufs=2))
            ppool = ctx.enter_context(tc.tile_pool(name="psum", bufs=1,
                                                   space="PSUM"))

            # ---- aux loads
            planes = persist.tile([128, 5 * NT * PAD], BF16)
            nc.sync.dma_start(out=planes[:], in_=planes_d[:])
            consts = persist.tile([128, CN], F32)
            nc.scalar.dma_start(out=consts[:], in_=consts_d[:])

            def plane(j):           # [128, NT, PAD]
                return (planes[:, j * NT * PAD:(j + 1) * NT * PAD]
                        .rearrange("p (t d) -> p t d", d=PAD))

            def plane_g(j, g):      # [128, GS, PAD] for group g
                c0 = j * NT * PAD + g * GS * PAD
                return (planes[:, c0:c0 + GS * PAD]
                        .rearrange("p (t d) -> p t d", d=PAD))

            x0 = consts[:, 0:NT]
            x1 = consts[:, NT:2 * NT]

            def phic_ap(l, j):      # [128,1]
                c = 2 * NT + l * 15 + j
                return consts[:, c:c + 1]

            def pqc_ap(l, qi, j):   # [128,1]
                c = 2 * NT + 15 * L + l * 16 + 8 * qi + j
                return consts[:, c:c + 1]

            bfc2 = consts[:, 2 * NT + 31 * L:2 * NT + 31 * L + NB]

            # ---- phi planes [128, NT] per (l, j<5):  phi = x@A + c
            phi = persist.tile([128, L * 5 * NT], F32)

            def phiv(l, j):
                c0 = (l * 5 + j) * NT
                return phi[:, c0:c0 + NT]

            for l in range(L):
                for j in range(5):
                    nc.vector.scalar_tensor_tensor(
                        out=phiv(l, j), in0=x1, scalar=phic_ap(l, 5 + j),
                        in1=phic_ap(l, 10 + j).to_broadcast([128, NT]),
                        op0=ALU.mult, op1=ALU.add)
                    nc.vector.scalar_tensor_tensor(
                        out=phiv(l, j), in0=x0, scalar=phic_ap(l, j),
                        in1=phiv(l, j), op0=ALU.mult, op1=ALU.add)

            # ---- mask+p4 planes per layer:  mnegl = mask + phi4 (bcast)
            mnegl = persist.tile([128, L * NT * PAD], BF16)

            def mnegl_ap(l):        # [128, NT, PAD]
                return (mnegl[:, l * NT * PAD:(l + 1) * NT * PAD]
                        .rearrange("p (t d) -> p t d", d=PAD))

            for l in range(L):
                nc.vector.tensor_tensor(
                    out=mnegl_ap(l), in0=plane(4),
                    in1=phiv(l, 4).to_broadcast([128, NT, PAD]), op=ALU.add)

            # ---- persistent state
            s_sb = persist.tile([128, L * NT], BF16)     # softmax denominators
            rr_sb = persist.tile([128, 4 * L * NT], BF16)  # weighted sums
            recs = persist.tile([128, L * NT], F32)
            ws = persist.tile([128, L * NT], F32)
            val_bf = persist.tile([128, KT], BF16)
            # one accumulation group per psum bank: start=True zeroes the
            # whole 2KB bank, so a bank hosts 2 output blocks with a single
            # start on its first matmul.
            pss = [ppool.tile([128, 2], F32, tag=f"ps{i}", name=f"ps{i}")
                   for i in range(NB // 2)]

            # weight chunk queue schedule: the Activation queue carries no
            # chunks in the first half (its engine must stay free for the
            # exp instructions the DVE reduces depend on), then takes the
            # lion's share of the second half.
            S, P, A = ((nc.sync, wS, "wS"), (nc.gpsimd, wP, "wP"),
                       (nc.scalar, wA, "wA"))
            qsched = ([S, P, A] * 11)[:32]

            def chunk_dma(ch):
                q_eng, q_pool, q_tag = qsched[ch]
                wtile = q_pool.tile([128, KTPER * N], BF16, tag=q_tag)
                rows = wt_d[ch * KTPER * 128:(ch + 1) * KTPER * 128, :]
                q_eng.dma_start(
                    out=wtile[:].rearrange("p (c n) -> p c n", c=KTPER),
                    in_=rows.rearrange("(c p) n -> p c n", p=128))
                return wtile

            prefetched = {}

            for g in range(NG):
                # --- message passing for this group's GS node tiles
                for l in range(L):
                    acc = work.tile([128, GS * PAD], BF16, tag="acc")
                    for jtl in range(GS):
                        jt = g * GS + jtl
                        asl = acc[:, jtl * PAD:(jtl + 1) * PAD]
                        psl = lambda j: (
                            planes[:, (j * NT + jt) * PAD:
                                   (j * NT + jt + 1) * PAD])
                        msl = mnegl[:, (l * NT + jt) * PAD:
                                    (l * NT + jt + 1) * PAD]
                        nc.vector.scalar_tensor_tensor(
                            out=asl, in0=psl(0),
                            scalar=phiv(l, 0)[:, jt:jt + 1],
                            in1=msl, op0=ALU.mult, op1=ALU.add)
                        for j in range(1, 4):
                            nc.vector.scalar_tensor_tensor(
                                out=asl, in0=psl(j),
                                scalar=phiv(l, j)[:, jt:jt + 1],
                                in1=asl, op0=ALU.mult, op1=ALU.add)
                    a = work.tile([128, GS * PAD], BF16, tag="a")
                    nc.scalar.activation(out=a[:], in_=acc[:], func=ACTF.Exp,
                                         bias=0.0, scale=1.0)
                    a3 = a[:].rearrange("p (t d) -> p t d", d=PAD)
                    with nc.allow_low_precision(reason="2e-2 rel-err gate"):
                        nc.vector.tensor_reduce(
                            out=s_sb[:, l * NT + g * GS:l * NT + (g + 1) * GS],
                            in_=a3, axis=mybir.AxisListType.X, op=ALU.add)
                        for j in range(4):
                            tmp = work.tile([128, GS * PAD], BF16, tag="tmp")
                            nc.vector.tensor_tensor(
                                out=tmp[:].rearrange("p (t d) -> p t d", d=PAD),
                                in0=a3, in1=plane_g(j, g), op=ALU.mult)
                            nc.vector.tensor_reduce(
                                out=rr_sb[:, (l * 4 + j) * NT + g * GS:
                                          (l * 4 + j) * NT + (g + 1) * GS],
                                in_=tmp[:].rearrange("p (t d) -> p t d", d=PAD),
                                axis=mybir.AxisListType.X, op=ALU.add)

                # --- combine into val columns for this group
                h = g
                HS = GS
                hsl = slice(h * HS, (h + 1) * HS)
                for l in range(L):
                    ssl = s_sb[:, l * NT + h * HS:l * NT + (h + 1) * HS]
                    rsl = recs[:, l * NT + h * HS:l * NT + (h + 1) * HS]
                    wsl = ws[:, l * NT + h * HS:l * NT + (h + 1) * HS]
                    seps = work.tile([128, HS], F32, tag="seps")
                    nc.vector.tensor_scalar(out=seps[:], in0=ssl, scalar1=1e-16,
                                            scalar2=None, op0=ALU.add)
                    nc.vector.reciprocal(out=rsl, in_=seps[:])
                    nc.vector.tensor_scalar(out=wsl, in0=ssl, scalar1=0.0,
                                            scalar2=None, op0=ALU.is_gt)
                    for qi in range(2):
                        rr = lambda j: rr_sb[:, (l * 4 + j) * NT + h * HS:
                                             (l * 4 + j) * NT + (h + 1) * HS]
                        u = work.tile([128, HS], F32, tag="u")
                        nc.vector.tensor_scalar(
                            out=u[:], in0=rr(0), scalar1=pqc_ap(l, qi, 0),
                            scalar2=None, op0=ALU.mult)
                        for j in range(1, 4):
                            nc.vector.scalar_tensor_tensor(
                                out=u[:], in0=rr(j), scalar=pqc_ap(l, qi, j),
                                in1=u[:], op0=ALU.mult, op1=ALU.add)
                        nc.vector.tensor_tensor(out=u[:], in0=u[:], in1=rsl,
                                                op=ALU.mult)
                        nc.vector.scalar_tensor_tensor(
                            out=u[:], in0=wsl, scalar=pqc_ap(l, qi, 4),
                            in1=u[:], op0=ALU.mult, op1=ALU.add)
                        nc.vector.scalar_tensor_tensor(
                            out=u[:], in0=x0[:, hsl], scalar=pqc_ap(l, qi, 5),
                            in1=u[:], op0=ALU.mult, op1=ALU.add)
                        nc.vector.scalar_tensor_tensor(
                            out=u[:], in0=x1[:, hsl], scalar=pqc_ap(l, qi, 6),
                            in1=u[:], op0=ALU.mult, op1=ALU.add)
                        # val columns kt' = (jt*L + l)*2 + qi, jt in half
                        c0 = (h * HS * L + l) * 2 + qi
                        nc.vector.tensor_scalar(
                            out=val_bf[:, c0:c0 + (HS - 1) * 2 * L + 1:2 * L],
                            in0=u[:], scalar1=pqc_ap(l, qi, 7), scalar2=None,
                            op0=ALU.add)

                # --- GEMV for this group's ktiles (weights stationary)
                for ch in range(g * KPG // KTPER,
                                (g + 1) * KPG // KTPER):
                    wtile = prefetched.pop(ch, None)
                    if wtile is None:
                        wtile = chunk_dma(ch)
                    for ci in range(KTPER):
                        kt = ch * KTPER + ci
                        for nb in range(NB):
                            ti, co = nb // 2, nb % 2
                            nc.tensor.matmul(
                                out=pss[ti][:, co:co + 1],
                                lhsT=wtile[:, ci * N + nb * 128:
                                           ci * N + (nb + 1) * 128],
                                rhs=val_bf[:, kt:kt + 1],
                                start=(kt == 0 and co == 0),
                                stop=(kt == KT - 1 and co == 1),
                                skip_group_check=True)

            y_sb = persist.tile([128, NB], F32)
            for i in range(NB // 2):
                nc.vector.tensor_tensor(out=y_sb[:, 2 * i:2 * i + 2],
                                        in0=pss[i][:],
                                        in1=bfc2[:, 2 * i:2 * i + 2],
                                        op=ALU.add)
            nc.sync.dma_start(out=out_d[:], in_=y_sb[:])

            if debug:
                nc.sync.dma_start(out=dbg_phi[:], in_=phi[:])
                for nm, t_, w_ in (("dbg_s", s_sb, L * NT),
                                   ("dbg_rr", rr_sb, 4 * L * NT),
                                   ("dbg_val", val_bf, KT)):
                    f = persist.tile([128, w_], F32)
                    nc.vector.tensor_copy(out=f[:], in_=t_[:])
                    d = {"dbg_s": dbg_s, "dbg_rr": dbg_rr,
                         "dbg_val": dbg_val}[nm]
                    nc.sync.dma_start(out=d[:], in_=f[:])

    nc.finalize()
    return nc


def kernel(x, edge_attr, src_idx, tgt_idx, Wq, bq, Wk, bk, Wv, bv, We,
           Wskip, bskip, Wfc, bfc, _trace=False):
    args = [np.asarray(a) for a in
            (x, edge_attr, src_idx, tgt_idx, Wq, bq, Wk, bk, Wv, bv, We,
             Wskip, bskip, Wfc, bfc)]
    core_inputs, dims = _host_prep(*args)
    nc = _build_graph(dims)
    res = run_bass_kernel_spmd(nc, core_inputs, core_ids=list(range(8)),
                               trace=_trace)
    T, N = dims["T"], dims["N"]
    y = np.zeros((T, N, 2), np.float32)
    for t in range(T):
        halves = []
        for h in range(2):
            o = np.asarray(res.results[2 * t + h]["out"])   # [128, NB]
            halves.append(o.T.reshape(-1))                  # n = nb*128 + p
        y[t] = np.concatenate(halves).reshape(N, 2)
    LAST.clear()
    LAST.update(exec_time_ns=res.exec_time_ns, dims=dims)
    return y


# revision 34
# speedup vs baseline: 1.0437x; 1.0437x over previous
"""Trainium2 Bass kernel for the ACOPF bus-embedder GNN (TransformerConv x L
layers + per-type dense head), distributed over 8 NeuronCores.

Sharding: core c handles node type t = c // 2 and column-half h = c % 2 of the
per-type fc weight.  The message passing is graph-partitioned by target node
on the host (edges sorted by target, slotted to fixed degree PAD); each core
pair duplicates it, so no collectives are needed.

Key algebraic reduction (C == 2): with q = x@Wq+bq, k = xs@Wk+bk+ea@We,
v = xs@Wv+bv+ea@We, the attention logit  alpha_e = q[tgt]*k_e / sqrt(H)
collapses to   alpha_e = p0*xs0 + p1*xs1 + p2*ea0 + p3*ea1 + p4
with per-target scalars p_j affine in x[tgt] (weight products folded on the
host, the affine map evaluated on-device).  The message is likewise affine in
the softmax-weighted sums r_j = sum_e a_e*plane_j, so the per-edge pipeline is
a few fused multiply-adds plus one exp.

Device mapping:
  - edge planes (xs0/xs1/ea0/ea1/mask) are host-slotted and DMA'd in bf16.
  - alpha / exp / weighted reductions run on DVE + Act, batched per group of
    4 node tiles to amortize instruction overhead.
  - the fc GEMV y = Wfc @ val streams the bf16 weight matrix over all three
    DMA-capable queues (sync/HWDGE, scalar/HWDGE, gpsimd/SWDGE) concurrently
    and feeds the PE with the weights as the *stationary* operand (ldweights),
    val as the moving operand, accumulating [128,1] psum columns - the PE cost
    of a GEMV is set by weight ingest, and ldweights overlaps the stream.
"""

import numpy as np
import ml_dtypes

from concourse import bass, bacc, mybir
import concourse.tile as tile
from concourse.bass_utils import run_bass_kernel_spmd

F32 = mybir.dt.float32
BF16 = mybir.dt.bfloat16
NPBF = ml_dtypes.bfloat16
ALU = mybir.AluOpType
ACTF = mybir.ActivationFunctionType

LAST = {}  # filled with exec info for test harnesses

NEG_BIG = -1.0e30


def _host_prep(x, edge_attr, src_idx, tgt_idx, Wq, bq, Wk, bk, Wv, bv, We,
               Wskip, bskip, Wfc, bfc):
    """Build per-core input maps + dims. All index juggling happens here."""
    T, N, C = x.shape
    E = src_idx.shape[1]
    L, H = Wq.shape[1], Wq.shape[3]
    K = Wfc.shape[2]
    assert T == 4, "sharding assumes 4 node types x 2 col-halves = 8 cores"
    assert C == 2 and K == 2 * L * N, (C, K, L, N)
    assert N % 128 == 0
    NT = N // 128
    TN = T * N
    KT = K // 128

    x_all = np.asarray(x, np.float32).reshape(TN, C)

    # --- per-type graph partitioning (sort by target, slot to fixed degree)
    per_type = []
    degmax = 0
    for t in range(T):
        tgt = np.asarray(tgt_idx[t])
        order = np.argsort(tgt, kind="stable")
        src_s = np.asarray(src_idx[t])[order]
        tgt_s = tgt[order]
        ea_s = np.asarray(edge_attr[t], np.float32)[order]
        deg = np.bincount(tgt_s, minlength=N)
        degmax = max(degmax, int(deg.max()))
        per_type.append((src_s, tgt_s, ea_s, deg))

    PAD = int(max(16, -(-degmax // 8) * 8))  # round up to mult of 8

    half = H // 2
    core_inputs = []
    for t in range(T):
        src_s, tgt_s, ea_s, deg = per_type[t]
        starts = np.zeros(N + 1, np.int64)
        np.cumsum(deg, out=starts[1:])
        pos = np.arange(E) - starts[tgt_s]

        # planes [5][NT][128][PAD] -> [128, 5*NT*PAD] bf16
        # j: 0=xs0 1=xs1 2=ea0 3=ea1 4=mask(-BIG at empty slots)
        pls = np.zeros((N, PAD, 4), np.float32)
        xs_e = x_all[src_s]
        pls[tgt_s, pos, 0] = xs_e[:, 0]
        pls[tgt_s, pos, 1] = xs_e[:, 1]
        pls[tgt_s, pos, 2] = ea_s[:, 0]
        pls[tgt_s, pos, 3] = ea_s[:, 1]
        valid = np.zeros((N, PAD), bool)
        valid[tgt_s, pos] = True
        mneg = np.where(valid, 0.0, NEG_BIG).astype(np.float32)
        allp = np.concatenate([pls, mneg[:, :, None]], axis=2)   # [N, PAD, 5]
        planes = (allp.reshape(NT, 128, PAD, 5)
                  .transpose(1, 3, 0, 2)           # [128, 5, NT, PAD]
                  .reshape(128, 5 * NT * PAD)).astype(NPBF)

        xt = np.asarray(x[t], np.float32)
        xpl = (xt.reshape(NT, 128, 2).transpose(1, 2, 0)
               .reshape(128, 2 * NT))              # [128, (c, NT)] c-major

        # phi constants: alpha = (x@A + c) . [xs0,xs1,ea0,ea1,1] / sqrt(H)
        phic = np.zeros((L, 15), np.float32)
        pqc = np.zeros((L, 16), np.float32)
        s = 1.0 / np.sqrt(H)
        for l in range(L):
            wq, bq_ = np.asarray(Wq[t, l], np.float64), np.asarray(bq[t, l], np.float64)
            wk, bk_ = np.asarray(Wk[t, l], np.float64), np.asarray(bk[t, l], np.float64)
            wv, bv_ = np.asarray(Wv[t, l], np.float64), np.asarray(bv[t, l], np.float64)
            we = np.asarray(We[t, l], np.float64)
            wsk, bsk_ = np.asarray(Wskip[t, l], np.float64), np.asarray(bskip[t, l], np.float64)
            A = np.concatenate([wq @ wk.T, wq @ we.T, (wq @ bk_)[:, None]], 1) * s  # [2,5]
            cvec = np.concatenate([bq_ @ wk.T, bq_ @ we.T, [bq_ @ bk_]]) * s        # [5]
            phic[l, 0:5] = A[0]
            phic[l, 5:10] = A[1]
            phic[l, 10:15] = cvec
            for qi, sl in enumerate((slice(0, half), slice(half, H))):
                pqc[l, 8 * qi:8 * qi + 8] = np.concatenate([
                    wv[:, sl].sum(1), we[:, sl].sum(1), [bv_[sl].sum()],
                    wsk[:, sl].sum(1), [bsk_[sl].sum()]])

        # Wfc rows permuted to k' = ((jt*L + l)*2 + q)*128 + p so val columns
        # are produced tile-major (original k = l*2N + 256*jt + 2*p + q).
        wt_full = np.asarray(Wfc[t], np.float32).T            # [K, 2N], k order
        wt_full = (wt_full.reshape(L, NT, 128, 2, 2 * N)
                   .transpose(1, 0, 3, 2, 4)
                   .reshape(K, 2 * N))                        # k' order

        for h in range(2):
            # consts: [xpl 2NT][phic 15L][pqc 16L][bfc2 16]
            bfc2 = (np.asarray(bfc[t, h * N:(h + 1) * N], np.float32)
                    .reshape(NT, 128).T)                      # [128, nb]
            consts = np.concatenate([
                xpl,
                np.tile(phic.reshape(1, 15 * L), (128, 1)),
                np.tile(pqc.reshape(1, 16 * L), (128, 1)),
                bfc2,
            ], axis=1)
            core_inputs.append({
                "planes": np.ascontiguousarray(planes),
                "consts": np.ascontiguousarray(consts.astype(np.float32)),
                "wt": np.ascontiguousarray(
                    wt_full[:, h * N:(h + 1) * N]).astype(NPBF),
            })

    dims = dict(T=T, N=N, C=C, E=E, L=L, H=H, K=K, NT=NT, PAD=PAD, TN=TN,
                KT=KT)
    return core_inputs, dims


def _build_graph(dims, debug=False):
    N, L, K = dims["N"], dims["L"], dims["K"]
    NT, PAD, KT = dims["NT"], dims["PAD"], dims["KT"]
    NB = N // 128              # psum column blocks
    CN = 2 * NT + 31 * L + NB  # consts width
    GS = 4                     # node tiles per message-passing group
    NG = NT // GS              # number of groups
    KTPER = 4                  # k tiles per weight DMA chunk (2 MiB)
    NCH = KT // KTPER          # weight chunks
    KPG = KT // NG             # ktiles per group

    nc = bacc.Bacc()
    planes_d = nc.declare_dram_parameter("planes", [128, 5 * NT * PAD], BF16,
                                         isOutput=False)
    consts_d = nc.declare_dram_parameter("consts", [128, CN], F32,
                                         isOutput=False)
    wt_d = nc.declare_dram_parameter("wt", [K, N], BF16, isOutput=False)
    out_d = nc.declare_dram_parameter("out", [128, NB], F32, isOutput=True)
    if debug:
        dbg_phi = nc.declare_dram_parameter("dbg_phi", [128, L * 5 * NT], F32,
                                            isOutput=True)
        dbg_s = nc.declare_dram_parameter("dbg_s", [128, L * NT], F32,
                                          isOutput=True)
        dbg_rr = nc.declare_dram_parameter("dbg_rr", [128, 4 * L * NT], F32,
                                           isOutput=True)
        dbg_val = nc.declare_dram_parameter("dbg_val", [128, KT], F32,
                                            isOutput=True)

    with tile.TileContext(nc) as tc:
        import contextlib
        with contextlib.ExitStack() as ctx:
            persist = ctx.enter_context(tc.tile_pool(name="persist", bufs=1))
            wS = ctx.enter_context(tc.tile_pool(name="wS", bufs=3))
            wA = ctx.enter_context(tc.tile_pool(name="wA", bufs=3))
            wP = ctx.enter_context(tc.tile_pool(name="wP", bufs=3))
            work = ctx.enter_context(tc.tile_pool(name="work", bufs=2))
            ppool = ctx.enter_context(tc.tile_pool(name="psum", bufs=1,
                                                   space="PSUM"))

            # ---- aux loads
            planes = persist.tile([128, 5 * NT * PAD], BF16)
            nc.sync.dma_start(out=planes[:], in_=planes_d[:])
            consts = persist.tile([128, CN], F32)
            nc.scalar.dma_start(out=consts[:], in_=consts_d[:])

            def plane(j):           # [128, NT, PAD]
                return (planes[:, j * NT * PAD:(j + 1) * NT * PAD]
                        .rearrange("p (t d) -> p t d", d=PAD))

            def plane_g(j, g):      # [128, GS, PAD] for group g
                c0 = j * NT * PAD + g * GS * PAD
                return (planes[:, c0:c0 + GS * PAD]
                        .rearrange("p (t d) -> p t d", d=PAD))

            x0 = consts[:, 0:NT]
            x1 = consts[:, NT:2 * NT]

            def phic_ap(l, j):      # [128,1]
                c = 2 * NT + l * 15 + j
                return consts[:, c:c + 1]

            def pqc_ap(l, qi, j):   # [128,1]
                c = 2 * NT + 15 * L + l * 16 + 8 * qi + j
                return consts[:, c:c + 1]

            bfc2 = consts[:, 2 * NT + 31 * L:2 * NT + 31 * L + NB]

            # ---- phi planes [128, NT] per (l, j<5):  phi = x@A + c
            phi = persist.tile([128, L * 5 * NT], F32)

            def phiv(l, j):
                c0 = (l * 5 + j) * NT
                return phi[:, c0:c0 + NT]

            for l in range(L):
                for j in range(5):
                    nc.vector.scalar_tensor_tensor(
                        out=phiv(l, j), in0=x1, scalar=phic_ap(l, 5 + j),
                        in1=phic_ap(l, 10 + j).to_broadcast([128, NT]),
                        op0=ALU.mult, op1=ALU.add)
                    nc.vector.scalar_tensor_tensor(
                        out=phiv(l, j), in0=x0, scalar=phic_ap(l, j),
                        in1=phiv(l, j), op0=ALU.mult, op1=ALU.add)

            # ---- mask+p4 planes per layer:  mnegl = mask + phi4 (bcast)
            mnegl = persist.tile([128, L * NT * PAD], BF16)

            def mnegl_ap(l):        # [128, NT, PAD]
                return (mnegl[:, l * NT * PAD:(l + 1) * NT * PAD]
                        .rearrange("p (t d) -> p t d", d=PAD))

            for l in range(L):
                nc.vector.tensor_tensor(
                    out=mnegl_ap(l), in0=plane(4),
                    in1=phiv(l, 4).to_broadcast([128, NT, PAD]), op=ALU.add)

            # ---- persistent state
            s_sb = persist.tile([128, L * NT], F32)      # softmax denominators
            rr_sb = persist.tile([128, 4 * L * NT], BF16)  # weighted sums
            recs = persist.tile([128, L * NT], F32)
            ws = persist.tile([128, L * NT], F32)
            val_bf = persist.tile([128, KT], BF16)
            # one accumulation group per psum bank: start=True zeroes the
            # whole 2KB bank, so a bank hosts 2 output blocks with a single
            # start on its first matmul.
            pss = [ppool.tile([128, 2], F32, tag=f"ps{i}", name=f"ps{i}")
                   for i in range(NB // 2)]

            # weight chunk queue schedule: the Activation queue carries no
            # chunks in the first half (its engine must stay free for the
            # exp instructions the DVE reduces depend on), then takes the
            # lion's share of the second half.
            S, P, A = ((nc.sync, wS, "wS"), (nc.gpsimd, wP, "wP"),
                       (nc.scalar, wA, "wA"))
            qsched = ([S, P, A] * 11)[:32]

            def chunk_dma(ch):
                q_eng, q_pool, q_tag = qsched[ch]
                wtile = q_pool.tile([128, KTPER * N], BF16, tag=q_tag)
                rows = wt_d[ch * KTPER * 128:(ch + 1) * KTPER * 128, :]
                q_eng.dma_start(
                    out=wtile[:].rearrange("p (c n) -> p c n", c=KTPER),
                    in_=rows.rearrange("(c p) n -> p c n", p=128))
                return wtile

            prefetched = {}

            for g in range(NG):
                # --- message passing for this group's GS node tiles
                for l in range(L):
                    acc = work.tile([128, GS * PAD], BF16, tag="acc")
                    for jtl in range(GS):
                        jt = g * GS + jtl
                        asl = acc[:, jtl * PAD:(jtl + 1) * PAD]
                        psl = lambda j: (
                            planes[:, (j * NT + jt) * PAD:
                                   (j * NT + jt + 1) * PAD])
                        msl = mnegl[:, (l * NT + jt) * PAD:
                                    (l * NT + jt + 1) * PAD]
                        nc.vector.scalar_tensor_tensor(
                            out=asl, in0=psl(0),
                            scalar=phiv(l, 0)[:, jt:jt + 1],
                            in1=msl, op0=ALU.mult, op1=ALU.add)
                        for j in range(1, 4):
                            nc.vector.scalar_tensor_tensor(
                                out=asl, in0=psl(j),
                                scalar=phiv(l, j)[:, jt:jt + 1],
                                in1=asl, op0=ALU.mult, op1=ALU.add)
                    a = work.tile([128, GS * PAD], BF16, tag="a")
                    nc.scalar.activation(out=a[:], in_=acc[:], func=ACTF.Exp,
                                         bias=0.0, scale=1.0)
                    a3 = a[:].rearrange("p (t d) -> p t d", d=PAD)
                    with nc.allow_low_precision(reason="2e-2 rel-err gate"):
                        nc.vector.tensor_reduce(
                            out=s_sb[:, l * NT + g * GS:l * NT + (g + 1) * GS],
                            in_=a3, axis=mybir.AxisListType.X, op=ALU.add)
                        # all 4 weighted sums in one broadcast mult + one
                        # strided reduce (a is broadcast over the plane axis)
                        tmp = work.tile([128, 4 * GS * PAD], BF16, tag="tmp")
                        pl4 = (planes[:, :4 * NT * PAD]
                               .rearrange("p (j t d) -> p j t d", j=4, d=PAD)
                               [:, :, g * GS:(g + 1) * GS, :])
                        ab = (a[:].rearrange("p (j t d) -> p j t d", j=1,
                                             d=PAD)
                              .to_broadcast([128, 4, GS, PAD]))
                        nc.vector.tensor_tensor(
                            out=tmp[:].rearrange("p (j t d) -> p j t d",
                                                 j=4, d=PAD),
                            in0=pl4, in1=ab, op=ALU.mult)
                        rro = (rr_sb[:, l * 4 * NT:(l + 1) * 4 * NT]
                               .rearrange("p (j t) -> p j t", j=4)
                               [:, :, g * GS:(g + 1) * GS])
                        nc.vector.tensor_reduce(
                            out=rro,
                            in_=tmp[:].rearrange("p (j t d) -> p j t d",
                                                 j=4, d=PAD),
                            axis=mybir.AxisListType.X, op=ALU.add)

                # --- combine into val columns for this group
                h = g
                HS = GS
                hsl = slice(h * HS, (h + 1) * HS)
                for l in range(L):
                    ssl = s_sb[:, l * NT + h * HS:l * NT + (h + 1) * HS]
                    rsl = recs[:, l * NT + h * HS:l * NT + (h + 1) * HS]
                    wsl = ws[:, l * NT + h * HS:l * NT + (h + 1) * HS]
                    seps = work.tile([128, HS], F32, tag="seps")
                    nc.vector.tensor_scalar(out=seps[:], in0=ssl, scalar1=1e-16,
                                            scalar2=None, op0=ALU.add)
                    nc.vector.reciprocal(out=rsl, in_=seps[:])
                    nc.vector.tensor_scalar(out=wsl, in0=ssl, scalar1=0.0,
                                            scalar2=None, op0=ALU.is_gt)
                    for qi in range(2):
                        rr = lambda j: rr_sb[:, (l * 4 + j) * NT + h * HS:
                                             (l * 4 + j) * NT + (h + 1) * HS]
                        u = work.tile([128, HS], F32, tag="u")
                        nc.vector.tensor_scalar(
                            out=u[:], in0=rr(0), scalar1=pqc_ap(l, qi, 0),
                            scalar2=None, op0=ALU.mult)
                        for j in range(1, 4):
                            nc.vector.scalar_tensor_tensor(
                                out=u[:], in0=rr(j), scalar=pqc_ap(l, qi, j),
                                in1=u[:], op0=ALU.mult, op1=ALU.add)
                        nc.vector.tensor_tensor(out=u[:], in0=u[:], in1=rsl,
                                                op=ALU.mult)
                        nc.vector.scalar_tensor_tensor(
                            out=u[:], in0=wsl, scalar=pqc_ap(l, qi, 4),
                            in1=u[:], op0=ALU.mult, op1=ALU.add)
                        nc.vector.scalar_tensor_tensor(
                            out=u[:], in0=x0[:, hsl], scalar=pqc_ap(l, qi, 5),
                            in1=u[:], op0=ALU.mult, op1=ALU.add)
                        nc.vector.scalar_tensor_tensor(
                            out=u[:], in0=x1[:, hsl], scalar=pqc_ap(l, qi, 6),
                            in1=u[:], op0=ALU.mult, op1=ALU.add)
                        # val columns kt' = (jt*L + l)*2 + qi, jt in half
                        c0 = (h * HS * L + l) * 2 + qi
                        nc.vector.tensor_scalar(
                            out=val_bf[:, c0:c0 + (HS - 1) * 2 * L + 1:2 * L],
                            in0=u[:], scalar1=pqc_ap(l, qi, 7), scalar2=None,
                            op0=ALU.add)

                # --- GEMV for this group's ktiles (weights stationary)
                for ch in range(g * KPG // KTPER,
                                (g + 1) * KPG // KTPER):
                    wtile = prefetched.pop(ch, None)
                    if wtile is None:
                        wtile = chunk_dma(ch)
                    for ci in range(KTPER):
                        kt = ch * KTPER + ci
                        for nb in range(NB):
                            ti, co = nb // 2, nb % 2
                            nc.tensor.matmul(
                                out=pss[ti][:, co:co + 1],
                                lhsT=wtile[:, ci * N + nb * 128:
                                           ci * N + (nb + 1) * 128],
                                rhs=val_bf[:, kt:kt + 1],
                                start=(kt == 0 and co == 0),
                                stop=(kt == KT - 1 and co == 1),
                                skip_group_check=True)

            y_sb = persist.tile([128, NB], F32)
            for i in range(NB // 2):
                nc.vector.tensor_tensor(out=y_sb[:, 2 * i:2 * i + 2],
                                        in0=pss[i][:],
                                        in1=bfc2[:, 2 * i:2 * i + 2],
                                        op=ALU.add)
            nc.sync.dma_start(out=out_d[:], in_=y_sb[:])

            if debug:
                nc.sync.dma_start(out=dbg_phi[:], in_=phi[:])
                for nm, t_, w_ in (("dbg_s", s_sb, L * NT),
                                   ("dbg_rr", rr_sb, 4 * L * NT),
                                   ("dbg_val", val_bf, KT)):
                    f = persist.tile([128, w_], F32)
                    nc.vector.tensor_copy(out=f[:], in_=t_[:])
                    d = {"dbg_s": dbg_s, "dbg_rr": dbg_rr,
                         "dbg_val": dbg_val}[nm]
                    nc.sync.dma_start(out=d[:], in_=f[:])

    nc.finalize()
    return nc


def kernel(x, edge_attr, src_idx, tgt_idx, Wq, bq, Wk, bk, Wv, bv, We,
           Wskip, bskip, Wfc, bfc, _trace=False):
    args = [np.asarray(a) for a in
            (x, edge_attr, src_idx, tgt_idx, Wq, bq, Wk, bk, Wv, bv, We,
             Wskip, bskip, Wfc, bfc)]
    core_inputs, dims = _host_prep(*args)
    nc = _build_graph(dims)
    res = run_bass_kernel_spmd(nc, core_inputs, core_ids=list(range(8)),
                               trace=_trace)
    T, N = dims["T"], dims["N"]
    y = np.zeros((T, N, 2), np.float32)
    for t in range(T):
        halves = []
        for h in range(2):
            o = np.asarray(res.results[2 * t + h]["out"])   # [128, NB]
            halves.append(o.T.reshape(-1))                  # n = nb*128 + p
        y[t] = np.concatenate(halves).reshape(N, 2)
    LAST.clear()
    LAST.update(exec_time_ns=res.exec_time_ns, dims=dims)
    return y
